# revision 29
# baseline (speedup 1.0000x reference)
"""Causal multi-head attention block (GPT-style) for Trainium2, 8 NeuronCores.

Problem: x[4,2048,768] -> qkv = x@W_attn+b_attn -> 12-head causal attention
         -> y@W_proj+b_proj -> out[4,2048,768]   (fp32 I/O)

Sharding: 4 batches x 2 head-groups (6 heads each). c_attn column-sharded,
c_proj row-sharded over head groups; AllReduce(add) over core pairs after
c_proj. Core c = 2*b + g handles batch b, heads 6g..6g+5.

Per-core kernel (all matmuls bf16, fp32 accumulation):
  1. QKV^T = Wa_g^T @ x^T: Q^T,K^T [384,2048] col-tile-major; V [2048,384]
     stored head-strided with a ones-column per head ([128,390] tiles).
  2. Flash-style causal attention in transposed-score orientation:
     S^T[k,q] blocks via row-packed pair matmuls (K=64 head A rows 0-63 /
     head B rows 64-127), exp on ScalarE (PSUM->SBUF bf16, scale=1/8),
     diagonal-band masking via precomputed shifted-tril mask multiply,
     y_u^T accumulation via V_aug[128,65] matmuls (65th row = softmax
     normalizer n[q] thanks to the ones column).
  3. Normalize: r = 1/n (DVE), partition-broadcast r via DMA, y^T = y_u^T*r.
  4. proj: out_partial[s,768] = sum_pairs yT_pair^T @ Wp_pair, + b_proj.
  5. AllReduce(add) over {2b, 2b+1} in 512-row chunks, overlapped.

The walrus build here allows only one sync-wait per instruction; a post-pass
(legalize_waits) hoists extra waits onto single-wait NOPs.
"""
import numpy as np
import ml_dtypes

import concourse.bass as bass
import concourse.tile as tile
from concourse import mybir
from concourse.bass_utils import run_bass_kernel_spmd
from concourse import mybir as mb

BF16 = mybir.dt.bfloat16
F32 = mybir.dt.float32

B, S, D = 4, 2048, 768
H, HD = 12, 64
G = 2                 # head groups
HL = H // G           # heads per core = 6
DL = HL * HD          # local head dims = 384
NP = HL // 2          # head pairs per core = 3
P = 128
QT = 512              # q tile
KT = 128              # k tile
N_CORES = 8
NI = S // QT          # 4 q tiles
NS = S // P           # 16 s tiles
NDT = D // P          # 6 D tiles
USE_AR = True
REPEAT = 1  # emit the whole computation N times (for timing-slope calibration)


def _legalize_waits(nc):
    n_split = 0
    for f in nc.m.functions:
        for bb in f.blocks:
            insts = list(bb.instructions)
            out = []
            changed = False
            for inst in insts:
                si = inst.sync_info
                if si is not None:
                    waits = list(si.on_wait)
                    if len(waits) > 1:
                        for w in waits[:-1]:
                            nop = mb.InstNoOp(name=f"I-wsplit-{nc.next_id()}", ins=[], outs=[])
                            nop.engine = inst.engine
                            nop.sync_info = mb.SyncInfo(on_wait=[w], on_update=[])
                            out.append(nop)
                            n_split += 1
                        inst.sync_info = mb.SyncInfo(on_wait=[waits[-1]], on_update=list(si.on_update))
                        changed = True
                out.append(inst)
            if changed:
                bb.instructions = out
    return n_split


def _build():
    nc = bass.Bass("TRN2", target_bir_lowering=False, debug=False, num_devices=N_CORES)

    xT = nc.dram_tensor("xT", [D, S], BF16, kind="ExternalInput").ap()
    wa = nc.dram_tensor("wa", [D, 3 * DL], BF16, kind="ExternalInput").ap()
    ba = nc.dram_tensor("ba", [P, 9], F32, kind="ExternalInput").ap()
    bv = nc.dram_tensor("bv", [P, DL], F32, kind="ExternalInput").ap()
    wp = nc.dram_tensor("wp", [DL, D], BF16, kind="ExternalInput").ap()
    bp = nc.dram_tensor("bp", [P, D], F32, kind="ExternalInput").ap()
    msk = nc.dram_tensor("msk", [P, 896], BF16, kind="ExternalInput").ap()
    out = nc.dram_tensor("out", [S, D], F32, kind="ExternalOutput").ap()
    if USE_AR:
        ar_in = nc.dram_tensor("ar_in", [S, D], F32).ap()
        ar_out = nc.dram_tensor("ar_out", [S, D], F32).ap()

    with tile.TileContext(nc) as tc:
        with (
            tc.tile_pool(name="wgt", bufs=1) as wpool,
            tc.tile_pool(name="qkv", bufs=1) as qkvpool,
            tc.tile_pool(name="pt", bufs=4) as ptpool,
            tc.tile_pool(name="yt", bufs=2) as ytpool,
            tc.tile_pool(name="nrm", bufs=3) as nrmpool,
            tc.tile_pool(name="ob", bufs=3) as obpool,
            tc.tile_pool(name="scp", bufs=3, space="PSUM") as scpool,
            tc.tile_pool(name="avp", bufs=1, space="PSUM") as avpool,
        ):
            # ---- phase 0: load weights/constants ----
            wak = []
            for t in range(NDT):
                wt_sb = wpool.tile([P, 3 * DL], BF16, tag=f"wak{t}")
                nc.sync.dma_start(wt_sb[:], wa[bass.ts(t, P), :])
                wak.append(wt_sb)
            xk = []
            for t in range(NDT):
                xt_sb = wpool.tile([P, S], BF16, tag=f"xk{t}")
                nc.sync.dma_start(xt_sb[:, 0 : S // 2], xT[bass.ts(t, P), 0 : S // 2])
                xk.append(xt_sb)
            for t in range(NDT):
                nc.sync.dma_start(xk[t][:, S // 2 :], xT[bass.ts(t, P), S // 2 :])
            wpp = []
            for p in range(NP):
                wp_sb = wpool.tile([P, D], BF16, tag=f"wpp{p}")
                nc.sync.dma_start(wp_sb[:], wp[bass.ts(p, P), :])
                wpp.append(wp_sb)
            ba_sb = wpool.tile([P, 9], F32, tag="ba")
            nc.sync.dma_start(ba_sb[:], ba[:])
            bv_sb = wpool.tile([P, DL], F32, tag="bv")
            nc.sync.dma_start(bv_sb[:], bv[:])
            bp_sb = wpool.tile([P, D], F32, tag="bp")
            nc.sync.dma_start(bp_sb[:], bp[:])
            msk_sb = wpool.tile([P, 896], BF16, tag="msk")
            nc.sync.dma_start(msk_sb[:], msk[:])
            ones_sb = wpool.tile([1, HD], mybir.dt.float32r, tag="ones")
            nc.vector.memset(ones_sb[:].bitcast(F32), 1.0)

            # ---- phase 1: Q^T, K^T  (col-tile m: 0-2 = Q pairs, 3-5 = K pairs)
            qt_t, kt_t = [None] * NP, [None] * NP

            def emit_qk(m, n2):
                if n2 == 0:
                    dst = qkvpool.tile([P, S], BF16, tag=f"qkvT{m}")
                    if m < NP:
                        qt_t[m] = dst
                    else:
                        kt_t[m - NP] = dst
                else:
                    dst = qt_t[m] if m < NP else kt_t[m - NP]
                ps = scpool.tile([P, 1024], F32, tag="sc")
                for half in range(2):
                    n = 2 * n2 + half
                    for t in range(NDT):
                        nc.tensor.matmul(
                            ps[:, bass.ts(half, QT)],
                            lhsT=wak[t][:, bass.ts(m, P)],
                            rhs=xk[t][:, bass.ts(n, QT)],
                            start=(t == 0),
                            stop=(t == NDT - 1),
                        )
                for half in range(2):
                    n = 2 * n2 + half
                    nc.vector.tensor_scalar_add(
                        dst[:, bass.ts(n, QT)],
                        ps[:, bass.ts(half, QT)],
                        ba_sb[:, m : m + 1],
                    )

            # V s-tiles, emitted lazily before the attention i-tile that needs them
            v_t = [None] * NS

            def emit_v(s):
                ps = scpool.tile([P, 1024], F32, tag="sc")
                for t in range(NDT):
                    nc.tensor.matmul(
                        ps[:, 0:DL],
                        lhsT=xk[t][:, bass.ts(s, P)],
                        rhs=wak[t][:, 2 * DL : 3 * DL],
                        start=(t == 0),
                        stop=(t == NDT - 1),
                    )
                vt = qkvpool.tile([P, HL * 65], BF16, tag=f"v{s}")
                vt3 = vt[:].rearrange("p (h x) -> p h x", h=HL)
                nc.vector.tensor_add(
                    vt3[:, :, 0:HD],
                    ps[:, 0:DL].rearrange("p (h x) -> p h x", h=HL),
                    bv_sb[:].rearrange("p (h x) -> p h x", h=HL),
                )
                nc.vector.memset(vt3[:, :, HD : HD + 1], 1.0)
                v_t[s] = vt

            yt_t = [None] * NP

            def emit_attn(i, p):
                njt = 4 * (i + 1)      # k tiles for this q tile
                noff = njt - 4         # full-width off-diagonal j tiles
                avA = avpool.tile([65, QT], F32, tag="avA")
                avB = avpool.tile([65, QT], F32, tag="avB")
                hA, hB = 2 * p, 2 * p + 1
                scs = {}
                # group list: full-width pairs of j, then 2 narrowed diagonal
                # groups (mi=0:N=512, mi=1:384 | mi=2:256, mi=3:128) -- the
                # diagonal band only has valid scores at q_local >= k + 128*mi
                groups = [("f", g) for g in range(noff // 2)] + [("d", 0), ("d", 1)]
                # (block j, N, q_offset_global, offset inside sc/pt tile)
                def blocks_of(kind, g):
                    if kind == "f":
                        return [(2 * g + jj, QT, i * QT, jj * QT) for jj in range(2)]
                    res = []
                    for idx in range(2):
                        mi = 2 * g + idx
                        n = QT - 128 * mi
                        off = 0 if idx == 0 else (QT if g == 0 else 256)
                        res.append((4 * i + mi, n, i * QT + 128 * mi, off))
                    return res

                def emit_scores(kind, g):
                    scA = scpool.tile([P, 1024], F32, tag="sc")
                    scB = scpool.tile([P, 1024], F32, tag="sc")
                    for j, n, q0, off in blocks_of(kind, g):
                        nc.tensor.matmul(
                            scA[:, off : off + n],
                            lhsT=kt_t[p][0:HD, bass.ts(j, P)],
                            rhs=qt_t[p][0:HD, q0 : q0 + n],
                            start=True, stop=True,
                        )
                        nc.tensor.matmul(
                            scB[:, off : off + n],
                            lhsT=kt_t[p][HD:P, bass.ts(j, P)],
                            rhs=qt_t[p][HD:P, q0 : q0 + n],
                            start=True, stop=True,
                        )
                    scs[(kind, g)] = (scA, scB)

                emit_scores(*groups[0])
                for gi, (kind, g) in enumerate(groups):
                    if gi + 1 < len(groups):
                        emit_scores(*groups[gi + 1])
                    scA, scB = scs.pop((kind, g))
                    width = sum(b[1] for b in blocks_of(kind, g))
                    woff = min(b[3] for b in blocks_of(kind, g))
                    ptA = ptpool.tile([P, 1024], BF16, tag="pt")
                    ptB = ptpool.tile([P, 1024], BF16, tag="pt")
                    nc.scalar.activation(ptA[:, woff : woff + width], scA[:, woff : woff + width],
                                         mybir.ActivationFunctionType.Exp, scale=0.125)
                    nc.scalar.activation(ptB[:, woff : woff + width], scB[:, woff : woff + width],
                                         mybir.ActivationFunctionType.Exp, scale=0.125)
                    if kind == "d":
                        for j, n, q0, off in blocks_of(kind, g):
                            for pt in (ptA, ptB):
                                nc.vector.tensor_mul(
                                    pt[:, off : off + n],
                                    pt[:, off : off + n],
                                    msk_sb[:, 384 : 384 + n],
                                )
                    for j, n, q0, off in blocks_of(kind, g):
                        qoff = q0 - i * QT
                        nc.tensor.matmul(
                            avA[:, qoff : qoff + n],
                            lhsT=v_t[j][:, hA * 65 : (hA + 1) * 65],
                            rhs=ptA[:, off : off + n],
                            start=(j == 0), stop=(j == njt - 1),
                        )
                        nc.tensor.matmul(
                            avB[:, qoff : qoff + n],
                            lhsT=v_t[j][:, hB * 65 : (hB + 1) * 65],
                            rhs=ptB[:, off : off + n],
                            start=(j == 0), stop=(j == njt - 1),
                        )

                # normalize: yT_pair[0:64] = avA[0:64]/avA[64], [64:128] = avB/..
                # n-row -> SBUF, broadcast to 64 partitions via ones-matmul,
                # reciprocal PSUM->SBUF, then multiply.
                ytp = ytpool.tile([P, QT], BF16, tag=f"yt{p}")
                n2 = nrmpool.tile([1, 2 * QT], mybir.dt.float32r, tag="n2")
                nc.vector.tensor_copy(n2[:, 0:QT], avA[64:65, :])
                nc.vector.tensor_copy(n2[:, QT:], avB[64:65, :])
                nb = scpool.tile([P, 1024], F32, tag="sc")
                nc.tensor.matmul(nb[0:HD, 0:QT], lhsT=ones_sb[:], rhs=n2[:, 0:QT], start=True, stop=True)
                nc.tensor.matmul(nb[0:HD, QT:], lhsT=ones_sb[:], rhs=n2[:, QT:], start=True, stop=True)
                rb = nrmpool.tile([HD, 2 * QT], F32, tag="rb")
                nc.vector.reciprocal(rb[:], nb[0:HD, :])
                nc.vector.tensor_mul(ytp[0:HD, :], avA[0:HD, :], rb[:, 0:QT])
                tmpB = nrmpool.tile([HD, QT], BF16, tag="tmpB")
                nc.vector.tensor_mul(tmpB[:], avB[0:HD, :], rb[:, QT:])
                nc.sync.dma_start(ytp[HD:P, :], tmpB[:])
                yt_t[p] = ytp

            # AllReduce chunk boundaries (rows): skewed so the last chunk is
            # tiny — its AR + final copy are the serial tail of the kernel.
            ar_chunks = []

            def emit_ar(r0, r1):
                nc.gpsimd.collective_compute(
                    "AllReduce",
                    mybir.AluOpType.add,
                    replica_groups=[[0, 1], [2, 3], [4, 5], [6, 7]],
                    ins=[ar_in[r0:r1, :].opt()],
                    outs=[ar_out[r0:r1, :].opt()],
                )

            def emit_out_copies():
                # final output copies, deferred to the kernel tail so they
                # don't contend with latency-critical mid-kernel DMAs;
                # bounced through SBUF (model prices DRAM->DRAM far above
                # two SBUF hops)
                for r in range(0, S, P):
                    oc = obpool.tile([P, D], F32, tag="oc")
                    nc.sync.dma_start(oc[:], ar_out[r : r + P, :])
                    nc.sync.dma_start(out[r : r + P, :], oc[:])

            def emit_proj(i):
                for ss in range(QT // P):
                    ps = scpool.tile([P, 1024], F32, tag="sc")
                    for p in range(NP):
                        nc.tensor.matmul(
                            ps[:, 0:512],
                            lhsT=yt_t[p][:, bass.ts(ss, P)],
                            rhs=wpp[p][:, 0:512],
                            start=(p == 0), stop=(p == NP - 1),
                        )
                        nc.tensor.matmul(
                            ps[:, 512:768],
                            lhsT=yt_t[p][:, bass.ts(ss, P)],
                            rhs=wpp[p][:, 512:768],
                            start=(p == 0), stop=(p == NP - 1),
                        )
                    ob = obpool.tile([P, D], F32, tag="ob")
                    nc.vector.tensor_add(ob[:], ps[:, 0:D], bp_sb[:])
                    row0 = i * QT + ss * P
                    dst = ar_in if USE_AR else out
                    nc.sync.dma_start(dst[row0 : row0 + P, :], ob[:])
                    if USE_AR:
                        done = row0 + P
                        while ar_chunks and ar_chunks[0][1] <= done:
                            r0, r1 = ar_chunks.pop(0)
                            emit_ar(r0, r1)

            # ---- main interleaved schedule ----
            for _rep in range(REPEAT):
                ar_chunks.clear()
                ar_chunks.extend([(0, 512), (512, 1024), (1024, 1536), (1536, 2048)])
                for m in range(2 * NP):
                    emit_qk(m, 0)
                    emit_qk(m, 1)
                for s in range(4):
                    emit_v(s)
                for i in range(NI):
                    for p in range(NP):
                        emit_attn(i, p)
                    if i + 1 < NI:
                        for s in range(4 * (i + 1), 4 * (i + 2)):
                            emit_v(s)
                    emit_proj(i)
                emit_out_copies()

    _legalize_waits(nc)
    return nc


_NC_CACHE = {}


def _get_nc():
    if "nc" not in _NC_CACHE:
        _NC_CACHE["nc"] = _build()
    return _NC_CACHE["nc"]


def _prep_inputs(x, W_attn, b_attn, W_proj, b_proj):
    bf = ml_dtypes.bfloat16
    x = np.asarray(x, np.float32)
    W_attn = np.asarray(W_attn, np.float32)
    b_attn = np.asarray(b_attn, np.float32)
    W_proj = np.asarray(W_proj, np.float32)
    b_proj = np.asarray(b_proj, np.float32)

    t_idx = np.arange(896)[None, :]
    k_idx = np.arange(P)[:, None]
    mask = (t_idx >= k_idx + 384).astype(bf)

    in_maps = []
    for c in range(N_CORES):
        b, g = divmod(c, 2)
        cols = slice(DL * g, DL * g + DL)
        xT = np.ascontiguousarray(x[b].T).astype(bf)
        wa = np.concatenate(
            [W_attn[:, 0:D][:, cols], W_attn[:, D : 2 * D][:, cols], W_attn[:, 2 * D :][:, cols]],
            axis=1,
        ).astype(bf)
        ba_sl = np.concatenate(
            [b_attn[0:D][cols], b_attn[D : 2 * D][cols], b_attn[2 * D :][cols]]
        ).astype(np.float32)
        ba2 = np.ascontiguousarray(ba_sl[: 2 * DL].reshape(6, P).T)
        ba9 = np.zeros((P, 9), np.float32)
        ba9[:, :6] = ba2
        bv_b = np.ascontiguousarray(np.broadcast_to(ba_sl[2 * DL :], (P, DL))).astype(np.float32)
        wp_c = np.ascontiguousarray(W_proj[cols, :]).astype(bf)
        bp_full = b_proj if g == 0 else np.zeros_like(b_proj)
        bp_b = np.ascontiguousarray(np.broadcast_to(bp_full, (P, D))).astype(np.float32)
        in_maps.append(
            {
                "xT": xT,
                "wa": wa,
                "ba": ba9,
                "bv": bv_b,
                "wp": wp_c,
                "bp": bp_b,
                "msk": mask,
            }
        )
    return in_maps


def kernel(x, W_attn, b_attn, W_proj, b_proj):
    in_maps = _prep_inputs(x, W_attn, b_attn, W_proj, b_proj)
    nc = _get_nc()
    res = run_bass_kernel_spmd(nc, in_maps, list(range(N_CORES)))
    if USE_AR:
        out = np.stack([res.results[2 * b]["out"] for b in range(B)])
    else:
        out = np.stack(
            [res.results[2 * b]["out"] + res.results[2 * b + 1]["out"] for b in range(B)]
        )
    return out.astype(np.float32)
